# revision 24
# baseline (speedup 1.0000x reference)
"""Trainium2 Bass kernel for a Conv-TasNet-style decoder (mask * wave ->
overlap_and_add -> trim).

Reference computation (per batch element b):
    A[c, d, t] = x[b, c, d, t] * x_wave[b, d, t]          (broadcast over c)
    frames     = A transposed to [c, t, d]  (frame length D=16, hop 8)
    unsliced   = overlap_and_add(frames, 8)               # [c, (T+1)*8]
    y          = unsliced[:, pad_left : -pad_right]

With hop=8 and D=16, overlap_and_add decomposes into two interleaved
streams, and for the middle region (everything when pad_left =
pad_right = 8):

    y[c][8s + r] = x[c, r, s+1]*w[r, s+1] + x[c, r+8, s]*w[r+8, s]

i.e. purely elementwise over s plus an 8-way interleave.  The device
kernel computes this on a [128 partitions x 8000] grid (partition p
owns frames [p*1000, (p+1)*1000)); the +1 frame shift is baked into
the DMA-load access patterns (flat-offset views).

The whole device pipeline runs in bfloat16: the kernel is DMA-engine
bound (16 DMA engines at ~23 GB/s each at 2 KB packets), so halving
the bytes moved halves the roofline; the bf16 rounding of
inputs/products/output is ~3e-3 relative error, inside the 2e-2 gate.
Inputs are downcast and the output upcast on the host.

Engine placement (measured on HW): GpSimd tensor ops stall concurrent
DVE ops ~7x, so GpSimd only issues DMAs (SWDGE ring).  Each DMA ring's
descriptor feed caps at ~165 GB/s, so the ~16.4 MB of traffic is
balanced across all three rings (SP: x-spk0-low + W-low; ACT:
x-spk0-high + W-high; SWDGE: x-spk1 both sides; stores spread over all
three behind their loads).  Products are contiguous bf16 DVE muls (2x
rate).  The 8-way interleave (r, j) -> 8j + r is a strided DVE add for
the head/tail sub-chunks and a contiguous DVE add + strided ACT copy
for the middle ones, splitting the permutation cost across engines.

Sharding: pure data parallel - core b computes batch element b (B=8
matches the 8 NeuronCores); no cross-core communication.
"""

import numpy as np

_B, _C, _D, _T = 8, 2, 16, 128000
_HOP = 8
_S = _T * _HOP            # padded per-speaker device output length (1024000)
_MID = _S - _HOP          # valid middle length (1023992)
_P = 128                  # SBUF partitions
_JB = _T // _P            # frames per partition block (1000)
_SC = 250                 # frames per partition per compute sub-chunk

_cached = None            # (nc, run_bass_kernel_spmd)


def _build():
    """Build the Bass module (one NeuronCore's program). Cached."""
    global _cached
    if _cached is not None:
        return _cached

    import bass_rust
    import concourse.bacc as bacc
    import concourse.mybir as mybir
    import concourse.tile as tile
    from concourse.bass_utils import run_bass_kernel_spmd

    bf16 = mybir.dt.bfloat16
    act_copy = bass_rust.ActivationFunctionType.Copy
    T, P = _T, _P

    nc = bacc.Bacc(debug=False)
    x = nc.declare_dram_parameter("x", [_C, _D, T], bf16, isOutput=False)
    w = nc.declare_dram_parameter("x_wave", [_D, T], bf16, isOutput=False)
    y = nc.declare_dram_parameter("y_pad", [_C, _S], bf16, isOutput=True)

    # Flat 1-D views let us bake the +1-frame shift into the AP offset
    # (a shifted [r, s] view crosses row boundaries, which plain
    # slice-then-rearrange cannot express).
    xf = x[:].rearrange("c d t -> (c d t)")
    wf = w[:].rearrange("d t -> (d t)")
    yf = y[:].rearrange("c n -> (c n)")

    def rpj(flat, start):
        # [p, r, j] view: element = flat[start + r*T + p*JB + j]
        return flat[start : start + 8 * T].rearrange("(r p j) -> p r j", r=8, p=P)

    # Store ring per global sub-chunk index (c*4 + k): stores alternate
    # between the two HWDGE rings, queued behind the (small) W loads;
    # the tail store rides SWDGE, which is idle by then, so the last
    # sub-chunk drains without queueing behind earlier stores.
    store_eng = ["sync", "scalar", "sync", "scalar", "sync", "scalar", "sync", "gpsimd"]

    with tile.TileContext(nc) as tc:
        with (
            tc.tile_pool(name="wpool", bufs=1) as wpool,
            tc.tile_pool(name="xpool", bufs=1) as xpool,
            tc.tile_pool(name="ppool", bufs=2) as ppool,
            tc.tile_pool(name="zpool", bufs=3) as zpool,
        ):
            def load(eng, pool, tag, view, n):
                t = pool.tile([P, 8, n], bf16, tag=tag, name=tag)
                eng.dma_start(out=t[:], in_=view)
                return t

            # The SWDGE ring coalesces descriptors of DRAM-contiguous
            # runs into ~6 KB packets (HWDGE keeps per-partition 2 KB
            # lines), which also earns it a proportionally larger share
            # of the shared DMA engines - so the x bulk (8.2 MB) rides
            # SWDGE while the small W loads (2 MB/side, needed first)
            # land quickly on the two otherwise-empty HWDGE rings.
            xlv0, xhv0 = rpj(xf, 1), rpj(xf, 8 * T)
            base1 = _D * T
            xlv1, xhv1 = rpj(xf, base1 + 1), rpj(xf, base1 + 8 * T)
            wlv, whv = rpj(wf, 1), rpj(wf, 8 * T)

            # Speaker 0's tranche is half-granular so the first compute
            # sub-chunks gate on ~half the bytes (SWDGE re-coalesces the
            # 1 KB lines into large packets, so the split is free);
            # speaker 1's stays whole-tile - its gate is the Q0 feed
            # tail either way.
            h0, h1 = np.s_[:, :, 0:500], np.s_[:, :, 500:1000]
            wl_tiles = [load(nc.sync, wpool, "wl0", wlv[h0], 500),
                        load(nc.sync, wpool, "wl1", wlv[h1], 500)]
            wh_tiles = [load(nc.scalar, wpool, "wh0", whv[h0], 500),
                        load(nc.scalar, wpool, "wh1", whv[h1], 500)]
            xl_tiles = {0: [load(nc.gpsimd, xpool, "xl0a", xlv0[h0], 500)], 1: []}
            xh_tiles = {0: [load(nc.gpsimd, xpool, "xh0a", xhv0[h0], 500)], 1: []}
            xl_tiles[0].append(load(nc.gpsimd, xpool, "xl0b", xlv0[h1], 500))
            xh_tiles[0].append(load(nc.gpsimd, xpool, "xh0b", xhv0[h1], 500))
            xl_tiles[1].append(load(nc.gpsimd, xpool, "xl1", xlv1[:], _JB))
            xh_tiles[1].append(load(nc.gpsimd, xpool, "xh1", xhv1[:], _JB))

            def tslice(tiles, j0):
                # 250-frame slice at j0 from a list of 500- or
                # 1000-frame tiles covering [0, 1000)
                n = tiles[0].shape[2]
                return tiles[j0 // n][:, :, j0 % n : j0 % n + _SC]

            for c in range(_C):
                y_c = yf[c * _S : (c + 1) * _S].rearrange("(p q) -> p q", p=P)
                # Speaker 1's low-side products are hoisted ahead of the
                # high-side ones: xl1 lands ~8 us before xh1 on the
                # SWDGE ring, so DVE runs these muls while xh1 is still
                # in flight instead of idling.
                hoisted = {}
                if c == 1:
                    for k in range(_JB // _SC):
                        j0 = k * _SC
                        h = ppool.tile([P, 8, _SC], bf16, tag=f"y1_{k}", name=f"y1_{k}")
                        nc.vector.tensor_mul(
                            h[:], tslice(xl_tiles[c], j0), tslice(wl_tiles, j0)
                        )
                        hoisted[k] = h
                for k in range(_JB // _SC):
                    j0 = k * _SC
                    if c == 1:
                        yt = hoisted[k]
                    else:
                        yt = ppool.tile([P, 8, _SC], bf16, tag="yt", name="yt")
                        nc.vector.tensor_mul(
                            yt[:], tslice(xl_tiles[c], j0), tslice(wl_tiles, j0)
                        )
                    tt = ppool.tile([P, 8, _SC], bf16, tag="tt", name="tt")
                    nc.vector.tensor_mul(tt[:], tslice(xh_tiles[c], j0), tslice(wh_tiles, j0))

                    # Interleaving add (r, j) -> 8j + r: strided reads,
                    # contiguous write.  The tail sub-chunk (c1k3) does
                    # it as a single strided DVE add (shortest serial
                    # chain after the last load); the rest add
                    # contiguously on DVE (2x bf16 rate) and interleave
                    # via a strided ACT copy so the two engines split
                    # the permutation cost.
                    zt = zpool.tile([P, 8 * _SC], bf16, tag="zt", name="zt")
                    if c == 1 and k == 3:
                        nc.vector.tensor_add(
                            zt[:],
                            yt.rearrange("p r j -> p j r"),
                            tt.rearrange("p r j -> p j r"),
                        )
                    else:
                        st = ppool.tile([P, 8, _SC], bf16, tag="st", name="st")
                        nc.vector.tensor_add(st[:], yt[:], tt[:])
                        nc.scalar.copy(zt[:], st.rearrange("p r j -> p j r"))
                    getattr(nc, store_eng[c * 4 + k]).dma_start(
                        out=y_c[:, 8 * j0 : 8 * (j0 + _SC)], in_=zt[:]
                    )

    nc.compile()  # legalize sync waits (>=1 wait/inst split into events)

    _cached = (nc, run_bass_kernel_spmd)
    return _cached


def _run_device(xb, wb, trace=False):
    nc, run_bass_kernel_spmd = _build()
    in_maps = [
        {"x": np.ascontiguousarray(xb[b]), "x_wave": np.ascontiguousarray(wb[b])}
        for b in range(_B)
    ]
    res = run_bass_kernel_spmd(nc, in_maps, core_ids=list(range(_B)), trace=trace)
    mid = np.stack(
        [r["y_pad"][:, :_MID].astype(np.float32) for r in res.results]
    )
    return mid, res


def kernel(x, x_wave, pad_left=8, pad_right=8, _trace=False, _return_res=False):
    import ml_dtypes

    x = np.asarray(x, dtype=np.float32)
    w = np.asarray(x_wave, dtype=np.float32)
    pl, pr = int(pad_left), int(pad_right)
    assert x.shape == (_B, _C, _D, _T) and w.shape == (_B, _D, _T)

    xb = x.astype(ml_dtypes.bfloat16)
    wb = w.astype(ml_dtypes.bfloat16)
    mid, res = _run_device(xb, wb, trace=_trace)

    if pl == 8 and pr == 8:
        out = mid
    else:
        # General trim: reconstruct the 8 leading / 8 trailing elements
        # of the unsliced overlap-add on the host (they only involve the
        # first/last frame, in full f32) and slice.
        front = x[:, :, 0:8, 0] * w[:, None, 0:8, 0]        # unsliced[0:8]
        back = x[:, :, 8:16, -1] * w[:, None, 8:16, -1]     # unsliced[-8:]
        full = np.concatenate([front, mid, back], axis=-1)  # [B, C, (T+1)*8]
        end = full.shape[-1] - pr
        out = np.ascontiguousarray(full[:, :, pl:end])

    if _return_res:
        return out, res
    return out


# revision 25
# speedup vs baseline: 1.0870x; 1.0870x over previous
"""Trainium2 Bass kernel for a Conv-TasNet-style decoder (mask * wave ->
overlap_and_add -> trim).

Reference computation (per batch element b):
    A[c, d, t] = x[b, c, d, t] * x_wave[b, d, t]          (broadcast over c)
    frames     = A transposed to [c, t, d]  (frame length D=16, hop 8)
    unsliced   = overlap_and_add(frames, 8)               # [c, (T+1)*8]
    y          = unsliced[:, pad_left : -pad_right]

With hop=8 and D=16, overlap_and_add decomposes into two interleaved
streams, and for the middle region (everything when pad_left =
pad_right = 8):

    y[c][8s + r] = x[c, r, s+1]*w[r, s+1] + x[c, r+8, s]*w[r+8, s]

i.e. purely elementwise over s plus an 8-way interleave.  The device
kernel computes this on a [128 partitions x 8000] grid (partition p
owns frames [p*1000, (p+1)*1000)); the +1 frame shift is baked into
the DMA-load access patterns (flat-offset views).

The whole device pipeline runs in bfloat16: the kernel is DMA-engine
bound (16 DMA engines at ~23 GB/s each at 2 KB packets), so halving
the bytes moved halves the roofline; the bf16 rounding of
inputs/products/output is ~3e-3 relative error, inside the 2e-2 gate.
Inputs are downcast and the output upcast on the host.

Engine placement (measured on HW): GpSimd tensor ops stall concurrent
DVE ops ~7x, so GpSimd only issues DMAs (SWDGE ring).  Each DMA ring's
descriptor feed caps at ~165 GB/s, so the ~16.4 MB of traffic is
balanced across all three rings (SP: x-spk0-low + W-low; ACT:
x-spk0-high + W-high; SWDGE: x-spk1 both sides; stores spread over all
three behind their loads).  Products are contiguous bf16 DVE muls (2x
rate).  The 8-way interleave (r, j) -> 8j + r is a strided DVE add for
the head/tail sub-chunks and a contiguous DVE add + strided ACT copy
for the middle ones, splitting the permutation cost across engines.

Sharding: pure data parallel - core b computes batch element b (B=8
matches the 8 NeuronCores); no cross-core communication.
"""

import numpy as np

_B, _C, _D, _T = 8, 2, 16, 128000
_HOP = 8
_S = _T * _HOP            # padded per-speaker device output length (1024000)
_MID = _S - _HOP          # valid middle length (1023992)
_P = 128                  # SBUF partitions
_JB = _T // _P            # frames per partition block (1000)
_SC = 250                 # frames per partition per compute sub-chunk

_cached = None            # (nc, run_bass_kernel_spmd)


def _build():
    """Build the Bass module (one NeuronCore's program). Cached."""
    global _cached
    if _cached is not None:
        return _cached

    import bass_rust
    import concourse.bacc as bacc
    import concourse.mybir as mybir
    import concourse.tile as tile
    from concourse.bass_utils import run_bass_kernel_spmd

    bf16 = mybir.dt.bfloat16
    act_copy = bass_rust.ActivationFunctionType.Copy
    T, P = _T, _P

    nc = bacc.Bacc(debug=False)
    x = nc.declare_dram_parameter("x", [_C, _D, T], bf16, isOutput=False)
    w = nc.declare_dram_parameter("x_wave", [_D, T], bf16, isOutput=False)
    y = nc.declare_dram_parameter("y_pad", [_C, _S], bf16, isOutput=True)

    # Flat 1-D views let us bake the +1-frame shift into the AP offset
    # (a shifted [r, s] view crosses row boundaries, which plain
    # slice-then-rearrange cannot express).
    xf = x[:].rearrange("c d t -> (c d t)")
    wf = w[:].rearrange("d t -> (d t)")
    yf = y[:].rearrange("c n -> (c n)")

    def rpj(flat, start):
        # [p, r, j] view: element = flat[start + r*T + p*JB + j]
        return flat[start : start + 8 * T].rearrange("(r p j) -> p r j", r=8, p=P)

    # Store ring per global sub-chunk index (c*4 + k): stores alternate
    # between the two HWDGE rings, queued behind the (small) W loads;
    # the tail store rides SWDGE, which is idle by then, so the last
    # sub-chunk drains without queueing behind earlier stores.
    store_eng = ["sync", "scalar", "sync", "scalar", "sync", "scalar", "sync", "gpsimd"]

    with tile.TileContext(nc) as tc:
        with (
            tc.tile_pool(name="wpool", bufs=1) as wpool,
            tc.tile_pool(name="xpool", bufs=1) as xpool,
            tc.tile_pool(name="ppool", bufs=2) as ppool,
            tc.tile_pool(name="zpool", bufs=3) as zpool,
        ):
            def load(eng, pool, tag, view, n):
                t = pool.tile([P, 8, n], bf16, tag=tag, name=tag)
                eng.dma_start(out=t[:], in_=view)
                return t

            # The SWDGE ring coalesces descriptors of DRAM-contiguous
            # runs into ~6 KB packets (HWDGE keeps per-partition 2 KB
            # lines), which also earns it a proportionally larger share
            # of the shared DMA engines - so the x bulk (8.2 MB) rides
            # SWDGE while the small W loads (2 MB/side, needed first)
            # land quickly on the two otherwise-empty HWDGE rings.
            xlv0, xhv0 = rpj(xf, 1), rpj(xf, 8 * T)
            base1 = _D * T
            xlv1, xhv1 = rpj(xf, base1 + 1), rpj(xf, base1 + 8 * T)
            wlv, whv = rpj(wf, 1), rpj(wf, 8 * T)

            wl = load(nc.sync, wpool, "wl", wlv[:], _JB)
            wh = load(nc.scalar, wpool, "wh", whv[:], _JB)
            xl0 = load(nc.gpsimd, xpool, "xl0", xlv0[:], _JB)
            xh0 = load(nc.gpsimd, xpool, "xh0", xhv0[:], _JB)
            xl1 = load(nc.gpsimd, xpool, "xl1", xlv1[:], _JB)
            xh1 = load(nc.gpsimd, xpool, "xh1", xhv1[:], _JB)

            wl_tiles, wh_tiles = [wl], [wh]
            xl_tiles = {0: [xl0], 1: [xl1]}
            xh_tiles = {0: [xh0], 1: [xh1]}

            def tslice(tiles, j0):
                # 250-frame slice at j0 from a list of 500- or
                # 1000-frame tiles covering [0, 1000)
                n = tiles[0].shape[2]
                return tiles[j0 // n][:, :, j0 % n : j0 % n + _SC]

            for c in range(_C):
                y_c = yf[c * _S : (c + 1) * _S].rearrange("(p q) -> p q", p=P)
                # Speaker 1's low-side products are hoisted ahead of the
                # high-side ones: xl1 lands ~8 us before xh1 on the
                # SWDGE ring, so DVE runs these muls while xh1 is still
                # in flight instead of idling.
                hoisted = {}
                if c == 1:
                    for k in range(_JB // _SC):
                        j0 = k * _SC
                        h = ppool.tile([P, 8, _SC], bf16, tag=f"y1_{k}", name=f"y1_{k}")
                        nc.vector.tensor_mul(
                            h[:], tslice(xl_tiles[c], j0), tslice(wl_tiles, j0)
                        )
                        hoisted[k] = h
                for k in range(_JB // _SC):
                    j0 = k * _SC
                    if c == 1:
                        yt = hoisted[k]
                    else:
                        yt = ppool.tile([P, 8, _SC], bf16, tag="yt", name="yt")
                        nc.vector.tensor_mul(
                            yt[:], tslice(xl_tiles[c], j0), tslice(wl_tiles, j0)
                        )
                    tt = ppool.tile([P, 8, _SC], bf16, tag="tt", name="tt")
                    nc.vector.tensor_mul(tt[:], tslice(xh_tiles[c], j0), tslice(wh_tiles, j0))

                    # Interleaving add (r, j) -> 8j + r: strided reads,
                    # contiguous write.  The tail sub-chunk (c1k3) does
                    # it as a single strided DVE add (shortest serial
                    # chain after the last load); the rest add
                    # contiguously on DVE (2x bf16 rate) and interleave
                    # via a strided ACT copy so the two engines split
                    # the permutation cost.
                    zt = zpool.tile([P, 8 * _SC], bf16, tag="zt", name="zt")
                    if c == 1 and k == 3:
                        nc.vector.tensor_add(
                            zt[:],
                            yt.rearrange("p r j -> p j r"),
                            tt.rearrange("p r j -> p j r"),
                        )
                    else:
                        st = ppool.tile([P, 8, _SC], bf16, tag="st", name="st")
                        nc.vector.tensor_add(st[:], yt[:], tt[:])
                        nc.scalar.copy(zt[:], st.rearrange("p r j -> p j r"))
                    getattr(nc, store_eng[c * 4 + k]).dma_start(
                        out=y_c[:, 8 * j0 : 8 * (j0 + _SC)], in_=zt[:]
                    )

    nc.compile()  # legalize sync waits (>=1 wait/inst split into events)

    _cached = (nc, run_bass_kernel_spmd)
    return _cached


def _run_device(xb, wb, trace=False):
    nc, run_bass_kernel_spmd = _build()
    in_maps = [
        {"x": np.ascontiguousarray(xb[b]), "x_wave": np.ascontiguousarray(wb[b])}
        for b in range(_B)
    ]
    res = run_bass_kernel_spmd(nc, in_maps, core_ids=list(range(_B)), trace=trace)
    mid = np.stack(
        [r["y_pad"][:, :_MID].astype(np.float32) for r in res.results]
    )
    return mid, res


def kernel(x, x_wave, pad_left=8, pad_right=8, _trace=False, _return_res=False):
    import ml_dtypes

    x = np.asarray(x, dtype=np.float32)
    w = np.asarray(x_wave, dtype=np.float32)
    pl, pr = int(pad_left), int(pad_right)
    assert x.shape == (_B, _C, _D, _T) and w.shape == (_B, _D, _T)

    xb = x.astype(ml_dtypes.bfloat16)
    wb = w.astype(ml_dtypes.bfloat16)
    mid, res = _run_device(xb, wb, trace=_trace)

    if pl == 8 and pr == 8:
        out = mid
    else:
        # General trim: reconstruct the 8 leading / 8 trailing elements
        # of the unsliced overlap-add on the host (they only involve the
        # first/last frame, in full f32) and slice.
        front = x[:, :, 0:8, 0] * w[:, None, 0:8, 0]        # unsliced[0:8]
        back = x[:, :, 8:16, -1] * w[:, None, 8:16, -1]     # unsliced[-8:]
        full = np.concatenate([front, mid, back], axis=-1)  # [B, C, (T+1)*8]
        end = full.shape[-1] - pr
        out = np.ascontiguousarray(full[:, :, pl:end])

    if _return_res:
        return out, res
    return out


# revision 26
# speedup vs baseline: 1.1316x; 1.0410x over previous
"""Trainium2 Bass kernel for a Conv-TasNet-style decoder (mask * wave ->
overlap_and_add -> trim).

Reference computation (per batch element b):
    A[c, d, t] = x[b, c, d, t] * x_wave[b, d, t]          (broadcast over c)
    frames     = A transposed to [c, t, d]  (frame length D=16, hop 8)
    unsliced   = overlap_and_add(frames, 8)               # [c, (T+1)*8]
    y          = unsliced[:, pad_left : -pad_right]

With hop=8 and D=16, overlap_and_add decomposes into two interleaved
streams, and for the middle region (everything when pad_left =
pad_right = 8):

    y[c][8s + r] = x[c, r, s+1]*w[r, s+1] + x[c, r+8, s]*w[r+8, s]

i.e. purely elementwise over s plus an 8-way interleave.  The device
kernel computes this on a [128 partitions x 8000] grid (partition p
owns frames [p*1000, (p+1)*1000)); the +1 frame shift is baked into
the DMA-load access patterns (flat-offset views).

The whole device pipeline runs in bfloat16: the kernel is DMA-engine
bound (16 DMA engines at ~23 GB/s each at 2 KB packets), so halving
the bytes moved halves the roofline; the bf16 rounding of
inputs/products/output is ~3e-3 relative error, inside the 2e-2 gate.
Inputs are downcast and the output upcast on the host.

Engine placement (measured on HW): GpSimd tensor ops stall concurrent
DVE ops ~7x, so GpSimd only issues DMAs (SWDGE ring).  Each DMA ring's
descriptor feed caps at ~165 GB/s, so the ~16.4 MB of traffic is
balanced across all three rings (SP: x-spk0-low + W-low; ACT:
x-spk0-high + W-high; SWDGE: x-spk1 both sides; stores spread over all
three behind their loads).  Products are contiguous bf16 DVE muls (2x
rate).  The 8-way interleave (r, j) -> 8j + r is a strided DVE add for
the head/tail sub-chunks and a contiguous DVE add + strided ACT copy
for the middle ones, splitting the permutation cost across engines.

Sharding: pure data parallel - core b computes batch element b (B=8
matches the 8 NeuronCores); no cross-core communication.
"""

import numpy as np

_B, _C, _D, _T = 8, 2, 16, 128000
_HOP = 8
_S = _T * _HOP            # padded per-speaker device output length (1024000)
_MID = _S - _HOP          # valid middle length (1023992)
_P = 128                  # SBUF partitions
_JB = _T // _P            # frames per partition block (1000)
_SC = 250                 # frames per partition per compute sub-chunk

_cached = None            # (nc, run_bass_kernel_spmd)


def _build():
    """Build the Bass module (one NeuronCore's program). Cached."""
    global _cached
    if _cached is not None:
        return _cached

    import bass_rust
    import concourse.bacc as bacc
    import concourse.mybir as mybir
    import concourse.tile as tile
    from concourse.bass_utils import run_bass_kernel_spmd

    bf16 = mybir.dt.bfloat16
    act_copy = bass_rust.ActivationFunctionType.Copy
    T, P = _T, _P

    nc = bacc.Bacc(debug=False)
    x = nc.declare_dram_parameter("x", [_C, _D, T], bf16, isOutput=False)
    w = nc.declare_dram_parameter("x_wave", [_D, T], bf16, isOutput=False)
    y = nc.declare_dram_parameter("y_pad", [_C, _S], bf16, isOutput=True)

    # Flat 1-D views let us bake the +1-frame shift into the AP offset
    # (a shifted [r, s] view crosses row boundaries, which plain
    # slice-then-rearrange cannot express).
    xf = x[:].rearrange("c d t -> (c d t)")
    wf = w[:].rearrange("d t -> (d t)")
    yf = y[:].rearrange("c n -> (c n)")

    def rpj(flat, start):
        # [p, r, j] view: element = flat[start + r*T + p*JB + j]
        return flat[start : start + 8 * T].rearrange("(r p j) -> p r j", r=8, p=P)

    # Store ring per global sub-chunk index (c*4 + k): stores alternate
    # between the two HWDGE rings, queued behind the (small) W loads;
    # the tail store rides SWDGE, which is idle by then, so the last
    # sub-chunk drains without queueing behind earlier stores.
    store_eng = ["sync", "scalar", "sync", "scalar", "sync", "scalar", "sync", "gpsimd"]

    with tile.TileContext(nc) as tc:
        with (
            tc.tile_pool(name="wpool", bufs=1) as wpool,
            tc.tile_pool(name="xpool", bufs=1) as xpool,
            tc.tile_pool(name="ppool", bufs=3) as ppool,
            tc.tile_pool(name="zpool", bufs=4) as zpool,
        ):
            def load(eng, pool, tag, view, n):
                t = pool.tile([P, 8, n], bf16, tag=tag, name=tag)
                eng.dma_start(out=t[:], in_=view)
                return t

            # The SWDGE ring coalesces descriptors of DRAM-contiguous
            # runs into ~6 KB packets (HWDGE keeps per-partition 2 KB
            # lines), which also earns it a proportionally larger share
            # of the shared DMA engines - so the x bulk (8.2 MB) rides
            # SWDGE while the small W loads (2 MB/side, needed first)
            # land quickly on the two otherwise-empty HWDGE rings.
            xlv0, xhv0 = rpj(xf, 1), rpj(xf, 8 * T)
            base1 = _D * T
            xlv1, xhv1 = rpj(xf, base1 + 1), rpj(xf, base1 + 8 * T)
            wlv, whv = rpj(wf, 1), rpj(wf, 8 * T)

            wl = load(nc.sync, wpool, "wl", wlv[:], _JB)
            wh = load(nc.scalar, wpool, "wh", whv[:], _JB)
            xl0 = load(nc.gpsimd, xpool, "xl0", xlv0[:], _JB)
            xh0 = load(nc.gpsimd, xpool, "xh0", xhv0[:], _JB)
            xl1 = load(nc.gpsimd, xpool, "xl1", xlv1[:], _JB)
            xh1 = load(nc.gpsimd, xpool, "xh1", xhv1[:], _JB)

            wl_tiles, wh_tiles = [wl], [wh]
            xl_tiles = {0: [xl0], 1: [xl1]}
            xh_tiles = {0: [xh0], 1: [xh1]}

            def tslice(tiles, j0):
                # 250-frame slice at j0 from a list of 500- or
                # 1000-frame tiles covering [0, 1000)
                n = tiles[0].shape[2]
                return tiles[j0 // n][:, :, j0 % n : j0 % n + _SC]

            for c in range(_C):
                y_c = yf[c * _S : (c + 1) * _S].rearrange("(p q) -> p q", p=P)
                # Speaker 1's low-side products are hoisted ahead of the
                # high-side ones: xl1 lands ~8 us before xh1 on the
                # SWDGE ring, so DVE runs these muls while xh1 is still
                # in flight instead of idling.
                hoisted = {}
                if c == 1:
                    for k in range(_JB // _SC):
                        j0 = k * _SC
                        h = ppool.tile([P, 8, _SC], bf16, tag=f"y1_{k}", name=f"y1_{k}")
                        nc.vector.tensor_mul(
                            h[:], tslice(xl_tiles[c], j0), tslice(wl_tiles, j0)
                        )
                        hoisted[k] = h
                for k in range(_JB // _SC):
                    j0 = k * _SC
                    if c == 1:
                        yt = hoisted[k]
                    else:
                        yt = ppool.tile([P, 8, _SC], bf16, tag="yt", name="yt")
                        nc.vector.tensor_mul(
                            yt[:], tslice(xl_tiles[c], j0), tslice(wl_tiles, j0)
                        )
                    tt = ppool.tile([P, 8, _SC], bf16, tag="tt", name="tt")
                    nc.vector.tensor_mul(tt[:], tslice(xh_tiles[c], j0), tslice(wh_tiles, j0))

                    # Interleaving add (r, j) -> 8j + r: strided reads,
                    # contiguous write.  The tail sub-chunk (c1k3) does
                    # it as a single strided DVE add (shortest serial
                    # chain after the last load); the rest add
                    # contiguously on DVE (2x bf16 rate) and interleave
                    # via a strided ACT copy so the two engines split
                    # the permutation cost.
                    zt = zpool.tile([P, 8 * _SC], bf16, tag="zt", name="zt")
                    if c == 1 and k == 3:
                        nc.vector.tensor_add(
                            zt[:],
                            yt.rearrange("p r j -> p j r"),
                            tt.rearrange("p r j -> p j r"),
                        )
                    else:
                        st = ppool.tile([P, 8, _SC], bf16, tag="st", name="st")
                        nc.vector.tensor_add(st[:], yt[:], tt[:])
                        nc.scalar.copy(zt[:], st.rearrange("p r j -> p j r"))
                    getattr(nc, store_eng[c * 4 + k]).dma_start(
                        out=y_c[:, 8 * j0 : 8 * (j0 + _SC)], in_=zt[:]
                    )

    nc.compile()  # legalize sync waits (>=1 wait/inst split into events)

    _cached = (nc, run_bass_kernel_spmd)
    return _cached


def _run_device(xb, wb, trace=False):
    nc, run_bass_kernel_spmd = _build()
    in_maps = [
        {"x": np.ascontiguousarray(xb[b]), "x_wave": np.ascontiguousarray(wb[b])}
        for b in range(_B)
    ]
    res = run_bass_kernel_spmd(nc, in_maps, core_ids=list(range(_B)), trace=trace)
    mid = np.stack(
        [r["y_pad"][:, :_MID].astype(np.float32) for r in res.results]
    )
    return mid, res


def kernel(x, x_wave, pad_left=8, pad_right=8, _trace=False, _return_res=False):
    import ml_dtypes

    x = np.asarray(x, dtype=np.float32)
    w = np.asarray(x_wave, dtype=np.float32)
    pl, pr = int(pad_left), int(pad_right)
    assert x.shape == (_B, _C, _D, _T) and w.shape == (_B, _D, _T)

    xb = x.astype(ml_dtypes.bfloat16)
    wb = w.astype(ml_dtypes.bfloat16)
    mid, res = _run_device(xb, wb, trace=_trace)

    if pl == 8 and pr == 8:
        out = mid
    else:
        # General trim: reconstruct the 8 leading / 8 trailing elements
        # of the unsliced overlap-add on the host (they only involve the
        # first/last frame, in full f32) and slice.
        front = x[:, :, 0:8, 0] * w[:, None, 0:8, 0]        # unsliced[0:8]
        back = x[:, :, 8:16, -1] * w[:, None, 8:16, -1]     # unsliced[-8:]
        full = np.concatenate([front, mid, back], axis=-1)  # [B, C, (T+1)*8]
        end = full.shape[-1] - pr
        out = np.ascontiguousarray(full[:, :, pl:end])

    if _return_res:
        return out, res
    return out


# revision 27
# speedup vs baseline: 1.1787x; 1.0416x over previous
"""Trainium2 Bass kernel for a Conv-TasNet-style decoder (mask * wave ->
overlap_and_add -> trim).

Reference computation (per batch element b):
    A[c, d, t] = x[b, c, d, t] * x_wave[b, d, t]          (broadcast over c)
    frames     = A transposed to [c, t, d]  (frame length D=16, hop 8)
    unsliced   = overlap_and_add(frames, 8)               # [c, (T+1)*8]
    y          = unsliced[:, pad_left : -pad_right]

With hop=8 and D=16, overlap_and_add decomposes into two interleaved
streams, and for the middle region (everything when pad_left =
pad_right = 8):

    y[c][8s + r] = x[c, r, s+1]*w[r, s+1] + x[c, r+8, s]*w[r+8, s]

i.e. purely elementwise over s plus an 8-way interleave.  The device
kernel computes this on a [128 partitions x 8000] grid (partition p
owns frames [p*1000, (p+1)*1000)); the +1 frame shift is baked into
the DMA-load access patterns (flat-offset views).

The whole device pipeline runs in bfloat16: the kernel is DMA-engine
bound (16 DMA engines at ~23 GB/s each at 2 KB packets), so halving
the bytes moved halves the roofline; the bf16 rounding of
inputs/products/output is ~3e-3 relative error, inside the 2e-2 gate.
Inputs are downcast and the output upcast on the host.

Engine placement (measured on HW): GpSimd tensor ops stall concurrent
DVE ops ~7x, so GpSimd only issues DMAs (SWDGE ring).  Each DMA ring's
descriptor feed caps at ~165 GB/s, so the ~16.4 MB of traffic is
balanced across all three rings (SP: x-spk0-low + W-low; ACT:
x-spk0-high + W-high; SWDGE: x-spk1 both sides; stores spread over all
three behind their loads).  Products are contiguous bf16 DVE muls (2x
rate).  The 8-way interleave (r, j) -> 8j + r is a strided DVE add for
the head/tail sub-chunks and a contiguous DVE add + strided ACT copy
for the middle ones, splitting the permutation cost across engines.

Sharding: pure data parallel - core b computes batch element b (B=8
matches the 8 NeuronCores); no cross-core communication.
"""

import numpy as np

_B, _C, _D, _T = 8, 2, 16, 128000
_HOP = 8
_S = _T * _HOP            # padded per-speaker device output length (1024000)
_MID = _S - _HOP          # valid middle length (1023992)
_P = 128                  # SBUF partitions
_JB = _T // _P            # frames per partition block (1000)
_SC = 250                 # frames per partition per compute sub-chunk

_cached = None            # (nc, run_bass_kernel_spmd)


def _build():
    """Build the Bass module (one NeuronCore's program). Cached."""
    global _cached
    if _cached is not None:
        return _cached

    import bass_rust
    import concourse.bacc as bacc
    import concourse.mybir as mybir
    import concourse.tile as tile
    from concourse.bass_utils import run_bass_kernel_spmd

    bf16 = mybir.dt.bfloat16
    act_copy = bass_rust.ActivationFunctionType.Copy
    T, P = _T, _P

    nc = bacc.Bacc(debug=False)
    x = nc.declare_dram_parameter("x", [_C, _D, T], bf16, isOutput=False)
    w = nc.declare_dram_parameter("x_wave", [_D, T], bf16, isOutput=False)
    y = nc.declare_dram_parameter("y_pad", [_C, _S], bf16, isOutput=True)

    # Flat 1-D views let us bake the +1-frame shift into the AP offset
    # (a shifted [r, s] view crosses row boundaries, which plain
    # slice-then-rearrange cannot express).
    xf = x[:].rearrange("c d t -> (c d t)")
    wf = w[:].rearrange("d t -> (d t)")
    yf = y[:].rearrange("c n -> (c n)")

    def rpj(flat, start):
        # [p, r, j] view: element = flat[start + r*T + p*JB + j]
        return flat[start : start + 8 * T].rearrange("(r p j) -> p r j", r=8, p=P)

    # Store ring per global sub-chunk index (c*4 + k): stores alternate
    # between the two HWDGE rings, queued behind the (small) W loads;
    # the tail store rides SWDGE, which is idle by then, so the last
    # sub-chunk drains without queueing behind earlier stores.
    store_eng = ["sync", "scalar", "sync", "scalar", "sync", "scalar", "sync", "gpsimd"]

    with tile.TileContext(nc) as tc:
        with (
            tc.tile_pool(name="wpool", bufs=1) as wpool,
            tc.tile_pool(name="xpool", bufs=1) as xpool,
            tc.tile_pool(name="ppool", bufs=3) as ppool,
            tc.tile_pool(name="zpool", bufs=4) as zpool,
        ):
            def load(eng, pool, tag, view, n):
                t = pool.tile([P, 8, n], bf16, tag=tag, name=tag)
                eng.dma_start(out=t[:], in_=view)
                return t

            # The SWDGE ring coalesces descriptors of DRAM-contiguous
            # runs into ~6 KB packets (HWDGE keeps per-partition 2 KB
            # lines), which also earns it a proportionally larger share
            # of the shared DMA engines - so the x bulk (8.2 MB) rides
            # SWDGE while the small W loads (2 MB/side, needed first)
            # land quickly on the two otherwise-empty HWDGE rings.
            xlv0, xhv0 = rpj(xf, 1), rpj(xf, 8 * T)
            base1 = _D * T
            xlv1, xhv1 = rpj(xf, base1 + 1), rpj(xf, base1 + 8 * T)
            wlv, whv = rpj(wf, 1), rpj(wf, 8 * T)

            wl = load(nc.sync, wpool, "wl", wlv[:], _JB)
            wh = load(nc.scalar, wpool, "wh", whv[:], _JB)
            xl0 = load(nc.gpsimd, xpool, "xl0", xlv0[:], _JB)
            xh0 = load(nc.gpsimd, xpool, "xh0", xhv0[:], _JB)
            xl1 = load(nc.gpsimd, xpool, "xl1", xlv1[:], _JB)
            xh1 = load(nc.gpsimd, xpool, "xh1", xhv1[:], _JB)

            wl_tiles, wh_tiles = [wl], [wh]
            xl_tiles = {0: [xl0], 1: [xl1]}
            xh_tiles = {0: [xh0], 1: [xh1]}

            def tslice(tiles, j0):
                # 250-frame slice at j0 from a list of 500- or
                # 1000-frame tiles covering [0, 1000)
                n = tiles[0].shape[2]
                return tiles[j0 // n][:, :, j0 % n : j0 % n + _SC]

            for c in range(_C):
                y_c = yf[c * _S : (c + 1) * _S].rearrange("(p q) -> p q", p=P)
                # Speaker 1's low-side products are hoisted ahead of the
                # high-side ones: xl1 lands ~8 us before xh1 on the
                # SWDGE ring, so DVE runs these muls while xh1 is still
                # in flight instead of idling.
                hoisted = {}
                if c == 1:
                    for k in range(_JB // _SC):
                        j0 = k * _SC
                        h = ppool.tile([P, 8, _SC], bf16, tag=f"y1_{k}", name=f"y1_{k}")
                        nc.vector.tensor_mul(
                            h[:], tslice(xl_tiles[c], j0), tslice(wl_tiles, j0)
                        )
                        hoisted[k] = h
                for k in range(_JB // _SC):
                    j0 = k * _SC
                    if c == 1:
                        yt = hoisted[k]
                    else:
                        yt = ppool.tile([P, 8, _SC], bf16, tag="yt", name="yt")
                        nc.vector.tensor_mul(
                            yt[:], tslice(xl_tiles[c], j0), tslice(wl_tiles, j0)
                        )
                    tt = ppool.tile([P, 8, _SC], bf16, tag="tt", name="tt")
                    nc.vector.tensor_mul(tt[:], tslice(xh_tiles[c], j0), tslice(wh_tiles, j0))

                    # Interleaving add (r, j) -> 8j + r: strided reads,
                    # contiguous write.  The tail sub-chunk (c1k3) does
                    # it as a single strided DVE add (shortest serial
                    # chain after the last load); the rest add
                    # contiguously on DVE (2x bf16 rate) and interleave
                    # via a strided ACT copy so the two engines split
                    # the permutation cost.
                    zt = zpool.tile([P, 8 * _SC], bf16, tag="zt", name="zt")
                    if c == 1 and k >= 2:
                        nc.vector.tensor_add(
                            zt[:],
                            yt.rearrange("p r j -> p j r"),
                            tt.rearrange("p r j -> p j r"),
                        )
                    else:
                        st = ppool.tile([P, 8, _SC], bf16, tag="st", name="st")
                        nc.vector.tensor_add(st[:], yt[:], tt[:])
                        nc.scalar.copy(zt[:], st.rearrange("p r j -> p j r"))
                    getattr(nc, store_eng[c * 4 + k]).dma_start(
                        out=y_c[:, 8 * j0 : 8 * (j0 + _SC)], in_=zt[:]
                    )

    nc.compile()  # legalize sync waits (>=1 wait/inst split into events)

    _cached = (nc, run_bass_kernel_spmd)
    return _cached


def _run_device(xb, wb, trace=False):
    nc, run_bass_kernel_spmd = _build()
    in_maps = [
        {"x": np.ascontiguousarray(xb[b]), "x_wave": np.ascontiguousarray(wb[b])}
        for b in range(_B)
    ]
    res = run_bass_kernel_spmd(nc, in_maps, core_ids=list(range(_B)), trace=trace)
    mid = np.stack(
        [r["y_pad"][:, :_MID].astype(np.float32) for r in res.results]
    )
    return mid, res


def kernel(x, x_wave, pad_left=8, pad_right=8, _trace=False, _return_res=False):
    import ml_dtypes

    x = np.asarray(x, dtype=np.float32)
    w = np.asarray(x_wave, dtype=np.float32)
    pl, pr = int(pad_left), int(pad_right)
    assert x.shape == (_B, _C, _D, _T) and w.shape == (_B, _D, _T)

    xb = x.astype(ml_dtypes.bfloat16)
    wb = w.astype(ml_dtypes.bfloat16)
    mid, res = _run_device(xb, wb, trace=_trace)

    if pl == 8 and pr == 8:
        out = mid
    else:
        # General trim: reconstruct the 8 leading / 8 trailing elements
        # of the unsliced overlap-add on the host (they only involve the
        # first/last frame, in full f32) and slice.
        front = x[:, :, 0:8, 0] * w[:, None, 0:8, 0]        # unsliced[0:8]
        back = x[:, :, 8:16, -1] * w[:, None, 8:16, -1]     # unsliced[-8:]
        full = np.concatenate([front, mid, back], axis=-1)  # [B, C, (T+1)*8]
        end = full.shape[-1] - pr
        out = np.ascontiguousarray(full[:, :, pl:end])

    if _return_res:
        return out, res
    return out
